# revision 25
# baseline (speedup 1.0000x reference)
"""Trainium2 Bass kernel for nn_DepthMemoryCache.

Reference computation (D=8, B=4, S=4096, C=1024, G=64):
    u     = einsum('bsc,gc->bsg', x[-1], W_u)
    keys  = einsum('dbc,gc->dbg', x.mean(2), W_u)
    gates = softmax(einsum('bsg,dbg->bsd', u, keys), axis=-1)
    out   = einsum('dbsc,bsd->bsc', x, gates)

Strategy: shard the sequence axis over 8 cores (core i gets
x[:, :, i*512:(i+1)*512, :]). Gates for batch b depend only on batch b's
means, so the kernel pipelines PER BATCH and reads HBM exactly once
(72MB/core total = 64 read + 8 write, vs 136MB for a two-pass scheme):

  A(b): stream the 8 depth slabs [512, C] once (16KB/partition
        descriptors via the (p j) row mapping), cast to a resident bf16
        SBUF cache on ACT, j-reduce each slab on DVE (bf16 2x fast mode)
        so the PE indicator-matmul sums see 4x fewer moving columns, and
        build uT = W @ x7^T on PE for d=7.
  AR(b): AllReduce the [G, D] partial keys for b (2KB) from GpSimd right
        after b's sums; latency hides under A(b+1)'s stream.
  B(b): logits via one small PE matmul per 128-row block, softmax on ACT,
        then per-j chains of scalar_tensor_tensor FMAs on DVE that
        accumulate in PSUM (one SBUF + one PSUM source = full DVE rate;
        two SBUF sources would halve it), final FMA lands in SBUF ystage
        and GpSimd writes y with 8KB/partition descriptors.

B(b-1) emission is interleaved into A(b)'s slab loop (prelude after slab
5, combine chains after slabs 6/7 and post-fixup) so in-order engine
queues never stall A(b)'s critical path: ACT never waits on combines,
and the AR trigger is gated only by ACT fixup copies, not DVE backlog.
SBUF cache = 2 generations x 8 slabs x [128, 4, 1024] bf16 = 16MB.
bf16 cache/means cost ~2e-3 relative error on the output.
"""
import sys

sys.path.insert(0, "/opt/trn_rl_repo")

from contextlib import ExitStack

import numpy as np
from concourse import bacc, bass, mybir, tile, masks
from concourse import bass_utils

F32 = mybir.dt.float32
BF16 = mybir.dt.bfloat16

D, B, S, C, G = 8, 4, 4096, 1024, 64
N_CORES = 8
P = 128                 # partition count / block rows
NKC = C // P            # 8 column chunks of 128
CV = 768                # combine column split: DVE gets [0:CV], GpSimd the rest


def build_body(tc, x, w, y, s_sh):
    """Emit the kernel IR. x:[D,B,s_sh,C], w:[G,C], y:[B,s_sh,C] dram APs."""
    nc = tc.nc
    nj = s_sh // P      # 4 row-chunks per partition
    mul, add = mybir.AluOpType.mult, mybir.AluOpType.add
    es = ExitStack()

    singles = es.enter_context(tc.tile_pool(name="singles", bufs=1))
    dram = es.enter_context(tc.tile_pool(name="dram", bufs=1, space="DRAM"))
    # warm-up AllReduce FIRST: the first cc trigger starts a ~43us barrier
    # and the cc stream serializes, so every microsecond earlier here pulls
    # AR(b0) earlier by the same amount
    ccw_in = dram.tile([1, 16], F32)
    ccw_out = dram.tile([1, 16], F32)
    warm_sb = singles.tile([1, 16], F32)
    nc.vector.memset(warm_sb[:], 0.0)
    nc.gpsimd.dma_start(ccw_in[:], warm_sb[:])
    nc.gpsimd.collective_compute(
        "AllReduce", add, replica_groups=[list(range(N_CORES))],
        ins=[ccw_in.opt()], outs=[ccw_out.opt()],
    )

    ident = singles.tile([P, P], F32)
    masks.make_identity(nc, ident[:])
    ident_bf = singles.tile([P, P], BF16)
    masks.make_identity(nc, ident_bf[:])
    # indicator stationaries: ind[:, r, m] = (m == r) / S — column-sums a
    # bf16 j-reduced slab into psum row r (r = depth index).
    ind_bf = singles.tile([P, D, D], BF16)
    nc.vector.memset(ind_bf[:], 0.0)
    for r in range(D):
        nc.vector.memset(ind_bf[:, r, r:r + 1], 1.0 / (N_CORES * s_sh))
    w_sb = singles.tile([G, C], F32)
    nc.sync.dma_start(w_sb[:], w[:])
    gates_sb = singles.tile([P, B, nj, D], F32)
    wT_sb = singles.tile([P, NKC, G], F32)
    wT_bf = singles.tile([P, NKC, G], BF16)
    keysT_sb = singles.tile([G, B, D], F32)
    keysT_bf = singles.tile([G, B, D], BF16)
    sumk_sb = singles.tile([G, B, D], F32)
    uT_sb = singles.tile([G, B, nj, P], BF16)

    # bf16 slab cache: 2 batch generations in flight (16 slabs x 8KB/part)
    cache = es.enter_context(tc.tile_pool(name="cache", bufs=2 * D))
    stage = es.enter_context(tc.tile_pool(name="stage", bufs=2))
    jsump = es.enter_context(tc.tile_pool(name="jsump", bufs=1))
    fxp = es.enter_context(tc.tile_pool(name="fxp", bufs=1))
    ppool = es.enter_context(tc.tile_pool(name="ppool", bufs=2))
    apool = es.enter_context(tc.tile_pool(name="apool", bufs=1))
    ystp = es.enter_context(tc.tile_pool(name="ystp", bufs=2))

    cc_in, cc_out = [], []
    for b in range(B):
        cc_in_b = dram.tile([G, D], F32, tag=f"ci{b}", name=f"cc_in_{b}")
        cc_out_b = dram.tile([G, D], F32, tag=f"co{b}", name=f"cc_out_{b}")
        cc_in.append(cc_in_b)
        cc_out.append(cc_out_b)

    psS = es.enter_context(tc.tile_pool(name="psumS", bufs=1, space="PSUM"))
    psT = es.enter_context(tc.tile_pool(name="psumT", bufs=2, space="PSUM"))
    psU = es.enter_context(tc.tile_pool(name="psumU", bufs=1, space="PSUM"))
    psF = es.enter_context(tc.tile_pool(name="psumF", bufs=1, space="PSUM"))
    psL = es.enter_context(tc.tile_pool(name="psumL", bufs=1, space="PSUM"))
    xtp = es.enter_context(tc.tile_pool(name="xtp", bufs=2))

    # one-time W_u transpose: wT[c, g] chunks (fp32 + bf16 copies)
    for k in range(NKC):
        tr = psF.tile([P, G], F32, tag="fix")
        nc.tensor.transpose(tr[:, :G], w_sb[:, k * P:(k + 1) * P], ident[:G, :G])
        nc.vector.tensor_copy(wT_sb[:, k, :], tr[:, :G])
        nc.scalar.copy(wT_bf[:, k, :], tr[:, :G])

    cache_tiles = {}            # (b, d) -> bf16 slab tile [P, nj, C]
    ystage = {}                 # (b, h) -> f32 tile [P, 2, C]

    def emit_slab(b, d, di, sums_ps):
        slab = stage.tile([P, nj, C], F32, tag="slab")
        nc.sync.dma_start(
            slab[:], x[d, b].rearrange("(p j) c -> p j c", j=nj))
        xbf = cache.tile([P, nj, C], BF16, tag="cslab")
        cache_tiles[(b, d)] = xbf
        # casts all on ACT (two [P, 2C] ops): DVE is reserved for combines
        for j in range(0, nj, 2):
            nc.scalar.copy(xbf[:, j:j + 2, :], slab[:, j:j + 2, :])
        # pair j-reduction on DVE (bf16 2x fast mode) halves the moving
        # columns PE must stream for the indicator column-sums
        jsum = jsump.tile([P, 2, C], BF16, tag="jsum")
        nc.vector.tensor_add(jsum[:, 0, :], xbf[:, 0, :], xbf[:, 1, :])
        nc.vector.tensor_add(jsum[:, 1, :], xbf[:, 2, :], xbf[:, 3, :])
        for h in range(2):
            for jj in range(2):
                nc.tensor.matmul(
                    sums_ps[:, h * 512:(h + 1) * 512],
                    ind_bf[:, d, :],
                    jsum[:, jj, h * 512:(h + 1) * 512],
                    start=(di == 0 and jj == 0),
                    stop=(di == D - 1 and jj == 1),
                )

    def emit_ublock(b, j):
        # uT[g, s-block] = sum_k (wT_k).T @ x7T_k on PE
        x7bf = cache_tiles[(b, D - 1)]
        u_ps = psU.tile([G, P], F32, tag="u")
        for k in range(NKC):
            xt_ps = psT.tile([P, P], BF16, tag="xt")
            nc.tensor.transpose(
                xt_ps[:], x7bf[:, j, k * P:(k + 1) * P], ident_bf[:])
            xt_sb = xtp.tile([P, P], BF16, tag="xt_sb")
            if k % 2 == 0:
                nc.scalar.copy(xt_sb[:], xt_ps[:])
            else:
                nc.vector.tensor_copy(xt_sb[:], xt_ps[:])
            nc.tensor.matmul(
                u_ps[:], wT_bf[:, k, :], xt_sb[:],
                start=(k == 0), stop=(k == NKC - 1))
        nc.scalar.copy(uT_sb[:, b, j, :], u_ps[:])

    def emit_fixup(b, sums_ps):
        # sums -> meanT chunks -> partial keysT; fixup copies on ACT so
        # the AR trigger is never gated by DVE combine backlog
        sums_sb = fxp.tile([D, C], F32, tag="sums_sb")
        nc.scalar.copy(sums_sb[:], sums_ps[:])
        mt_ps = psF.tile([P, NKC * D], F32, tag="fix")
        for k in range(NKC):
            nc.tensor.matmul(
                mt_ps[:, k * D:(k + 1) * D],
                sums_sb[:, k * P:(k + 1) * P], ident[:D, :D],
                is_transpose=True, start=(k == 0), stop=(k == NKC - 1))
        meanT_tmp = xtp.tile([P, NKC * D], F32, tag="mt")
        nc.scalar.copy(meanT_tmp[:], mt_ps[:])
        keys_ps = psF.tile([P, NKC * D], F32, tag="fix")
        for k in range(NKC):
            nc.tensor.matmul(
                keys_ps[:G, :D],
                wT_sb[:, k, :],
                meanT_tmp[:, k * D:(k + 1) * D],
                start=(k == 0), stop=(k == NKC - 1))
        nc.scalar.copy(sumk_sb[:, b, :], keys_ps[:G, :D])
        nc.gpsimd.dma_start(cc_in[b][:], sumk_sb[:, b, :])
        nc.gpsimd.collective_compute(
            "AllReduce", add, replica_groups=[list(range(N_CORES))],
            ins=[cc_in[b].opt()], outs=[cc_out[b].opt()],
        )

    def emit_prelude(b):
        # keysT fetch + logits + softmax for batch b
        nc.gpsimd.dma_start(keysT_sb[:, b, :], cc_out[b][:])
        nc.scalar.copy(keysT_bf[:, b, :], keysT_sb[:, b, :])
        for j in range(nj):
            lg_ps = psL.tile([P, D], F32, tag="lg")
            nc.tensor.matmul(lg_ps[:], uT_sb[:, b, j, :], keysT_bf[:, b, :])
            e_sb = xtp.tile([P, D], F32, tag="e")
            z_sb = xtp.tile([P, 1], F32, tag="z")
            rz_sb = xtp.tile([P, 1], F32, tag="rz")
            nc.scalar.activation(
                e_sb[:], lg_ps[:], mybir.ActivationFunctionType.Exp,
                accum_out=z_sb[:])
            nc.vector.reciprocal(rz_sb[:], z_sb[:])
            nc.scalar.mul(gates_sb[:, b, j, :], e_sb[:], rz_sb[:])

    def emit_chain(b, j):
        # weighted-depth combine for block j, all in bf16 so every op runs
        # in the DVE 2x fast mode (tensor_scalar / tensor_tensor; STT is
        # not fast-mode eligible): 8 products + 7 adds, final add emits
        # f32. bf16 accumulation costs ~0.5% relative — budget is 2e-2.
        # For the tail batch the products run on ACT (idle there), halving
        # the exposed tail combine.
        h, jj = j // 2, j % 2
        n_act = D - 1 if b == B - 1 else 2
        if jj == 0:
            yst_t = ystp.tile([P, 2, C], F32, tag="yst", name=f"yst_{b}_{h}")
            ystage[(b, h)] = yst_t
        yst = ystage[(b, h)]
        acc = apool.tile([P, C], BF16, tag="cacc")
        dorder = [D - 1] + list(range(D - 1))
        for di, d in enumerate(dorder):
            xbf = cache_tiles[(b, d)]
            g = gates_sb[:, b, j, d:d + 1]
            if di == 0:
                nc.vector.tensor_scalar_mul(acc[:], xbf[:, j, :], g)
                continue
            p = ppool.tile([P, C], BF16, tag="prod")
            if di <= n_act:
                nc.scalar.mul(p[:], xbf[:, j, :], g)
            else:
                nc.vector.tensor_scalar_mul(p[:], xbf[:, j, :], g)
            if di < D - 1:
                nc.vector.tensor_add(acc[:], acc[:], p[:])
            else:
                nc.vector.tensor_add(yst[:, jj, :], acc[:], p[:])
        if j == 3:
            for d in range(D):
                del cache_tiles[(b, d)]

    def emit_ywrite(b, h):
        nc.gpsimd.dma_start(
            y[b].rearrange("(p j) c -> p j c", j=nj)[:, 2 * h:2 * h + 2, :],
            ystage.pop((b, h))[:])

    dorder = [D - 1] + list(range(D - 1))
    for b in range(B):
        sums_ps = psS.tile([D, C], F32, tag="sums")
        for si, d in enumerate(dorder):
            emit_slab(b, d, si, sums_ps)
            if 1 <= si <= nj:
                emit_ublock(b, si - 1)
            if b >= 1:
                if si == 5:
                    emit_prelude(b - 1)
                elif si == 6:
                    emit_chain(b - 1, 0)
                    emit_chain(b - 1, 1)
                    emit_ywrite(b - 1, 0)
                elif si == 7:
                    emit_chain(b - 1, 2)
        emit_fixup(b, sums_ps)
        if b >= 1:
            emit_chain(b - 1, 3)
            emit_ywrite(b - 1, 1)
    emit_prelude(B - 1)
    for j in range(nj):
        emit_chain(B - 1, j)
        if j % 2 == 1:
            emit_ywrite(B - 1, j // 2)

    es.close()


def build_nc(s_sh):
    nc = bacc.Bacc("TRN2", target_bir_lowering=False, debug=False,
                   num_devices=N_CORES)
    x_ap = nc.dram_tensor("x", [D, B, s_sh, C], F32, kind="ExternalInput").ap()
    w_ap = nc.dram_tensor("w", [G, C], F32, kind="ExternalInput").ap()
    y_ap = nc.dram_tensor("y", [B, s_sh, C], F32, kind="ExternalOutput").ap()
    with tile.TileContext(nc) as tc:
        build_body(tc, x_ap, w_ap, y_ap, s_sh)
    nc.compile()
    return nc


_NC_CACHE = {}


def _get_nc(s_sh):
    if s_sh not in _NC_CACHE:
        _NC_CACHE[s_sh] = build_nc(s_sh)
    return _NC_CACHE[s_sh]


def run(cached_states, W_u, trace=False, trace_cores=None):
    s_sh = S // N_CORES
    nc = _get_nc(s_sh)
    xs = np.asarray(cached_states, dtype=np.float32)
    ws = np.ascontiguousarray(np.asarray(W_u, dtype=np.float32))
    in_maps = []
    for i in range(N_CORES):
        sh = np.ascontiguousarray(xs[:, :, i * s_sh:(i + 1) * s_sh, :])
        in_maps.append({"x": sh, "w": ws})
    res = bass_utils.run_bass_kernel_spmd(
        nc, in_maps, core_ids=list(range(N_CORES)), trace=trace,
        trace_cores=trace_cores)
    out = np.empty((B, S, C), np.float32)
    for i in range(N_CORES):
        out[:, i * s_sh:(i + 1) * s_sh, :] = res.results[i]["y"]
    return out, res


def kernel(cached_states, W_u):
    out, _ = run(cached_states, W_u)
    return out


# revision 26
# speedup vs baseline: 1.1100x; 1.1100x over previous
"""Trainium2 Bass kernel for nn_DepthMemoryCache.

Reference computation (D=8, B=4, S=4096, C=1024, G=64):
    u     = einsum('bsc,gc->bsg', x[-1], W_u)
    keys  = einsum('dbc,gc->dbg', x.mean(2), W_u)
    gates = softmax(einsum('bsg,dbg->bsd', u, keys), axis=-1)
    out   = einsum('dbsc,bsd->bsc', x, gates)

Strategy: shard the sequence axis over 8 cores (core i gets
x[:, :, i*512:(i+1)*512, :]). Gates for batch b depend only on batch b's
means, so the kernel pipelines PER BATCH and reads HBM exactly once
(72MB/core total = 64 read + 8 write, vs 136MB for a two-pass scheme):

  A(b): stream the 8 depth slabs [512, C] once (16KB/partition
        descriptors via the (p j) row mapping), cast to a resident bf16
        SBUF cache on ACT, j-reduce each slab on DVE (bf16 2x fast mode)
        so the PE indicator-matmul sums see 4x fewer moving columns, and
        build uT = W @ x7^T on PE for d=7.
  AR(b): AllReduce the [G, D] partial keys for b (2KB) from GpSimd right
        after b's sums; latency hides under A(b+1)'s stream.
  B(b): logits via one small PE matmul per 128-row block, softmax on ACT,
        then per-j chains of scalar_tensor_tensor FMAs on DVE that
        accumulate in PSUM (one SBUF + one PSUM source = full DVE rate;
        two SBUF sources would halve it), final FMA lands in SBUF ystage
        and GpSimd writes y with 8KB/partition descriptors.

B(b-1) emission is interleaved into A(b)'s slab loop (prelude after slab
5, combine chains after slabs 6/7 and post-fixup) so in-order engine
queues never stall A(b)'s critical path: ACT never waits on combines,
and the AR trigger is gated only by ACT fixup copies, not DVE backlog.
SBUF cache = 2 generations x 8 slabs x [128, 4, 1024] bf16 = 16MB.
bf16 cache/means cost ~2e-3 relative error on the output.
"""
import sys

sys.path.insert(0, "/opt/trn_rl_repo")

from contextlib import ExitStack

import numpy as np
from concourse import bacc, bass, mybir, tile, masks
from concourse import bass_utils

F32 = mybir.dt.float32
BF16 = mybir.dt.bfloat16

D, B, S, C, G = 8, 4, 4096, 1024, 64
N_CORES = 8
P = 128                 # partition count / block rows
NKC = C // P            # 8 column chunks of 128
CV = 768                # combine column split: DVE gets [0:CV], GpSimd the rest


def build_body(tc, x, w, y, s_sh):
    """Emit the kernel IR. x:[D,B,s_sh,C], w:[G,C], y:[B,s_sh,C] dram APs."""
    nc = tc.nc
    nj = s_sh // P      # 4 row-chunks per partition
    mul, add = mybir.AluOpType.mult, mybir.AluOpType.add
    es = ExitStack()

    singles = es.enter_context(tc.tile_pool(name="singles", bufs=1))
    dram = es.enter_context(tc.tile_pool(name="dram", bufs=1, space="DRAM"))
    # warm-up AllReduce FIRST: the first cc trigger starts a ~43us barrier
    # and the cc stream serializes, so every microsecond earlier here pulls
    # AR(b0) earlier by the same amount
    ccw_in = dram.tile([1, 16], F32)
    ccw_out = dram.tile([1, 16], F32)
    warm_sb = singles.tile([1, 16], F32)
    nc.vector.memset(warm_sb[:], 0.0)
    nc.gpsimd.dma_start(ccw_in[:], warm_sb[:])
    nc.gpsimd.collective_compute(
        "AllReduce", add, replica_groups=[list(range(N_CORES))],
        ins=[ccw_in.opt()], outs=[ccw_out.opt()],
    )

    ident = singles.tile([P, P], F32)
    masks.make_identity(nc, ident[:])
    ident_bf = singles.tile([P, P], BF16)
    masks.make_identity(nc, ident_bf[:])
    # indicator stationaries: ind[:, r, m] = (m == r) / S — column-sums a
    # bf16 j-reduced slab into psum row r (r = depth index).
    ind_bf = singles.tile([P, D, D], BF16)
    nc.vector.memset(ind_bf[:], 0.0)
    for r in range(D):
        nc.vector.memset(ind_bf[:, r, r:r + 1], 1.0 / (N_CORES * s_sh))
    w_sb = singles.tile([G, C], F32)
    nc.sync.dma_start(w_sb[:], w[:])
    gates_sb = singles.tile([P, B, nj, D], F32)
    wT_sb = singles.tile([P, NKC, G], F32)
    wT_bf = singles.tile([P, NKC, G], BF16)
    keysT_sb = singles.tile([G, B, D], F32)
    keysT_bf = singles.tile([G, B, D], BF16)
    sumk_sb = singles.tile([G, B, D], F32)
    uT_sb = singles.tile([G, B, nj, P], BF16)

    # bf16 slab cache: 2 batch generations in flight (16 slabs x 8KB/part)
    cache = es.enter_context(tc.tile_pool(name="cache", bufs=2 * D))
    stage = es.enter_context(tc.tile_pool(name="stage", bufs=2))
    jsump = es.enter_context(tc.tile_pool(name="jsump", bufs=1))
    fxp = es.enter_context(tc.tile_pool(name="fxp", bufs=1))
    ppool = es.enter_context(tc.tile_pool(name="ppool", bufs=2))
    apool = es.enter_context(tc.tile_pool(name="apool", bufs=1))
    ystp = es.enter_context(tc.tile_pool(name="ystp", bufs=2))

    cc_in, cc_out = [], []
    for b in range(B):
        cc_in_b = dram.tile([G, D], F32, tag=f"ci{b}", name=f"cc_in_{b}")
        cc_out_b = dram.tile([G, D], F32, tag=f"co{b}", name=f"cc_out_{b}")
        cc_in.append(cc_in_b)
        cc_out.append(cc_out_b)

    psS = es.enter_context(tc.tile_pool(name="psumS", bufs=1, space="PSUM"))
    psT = es.enter_context(tc.tile_pool(name="psumT", bufs=2, space="PSUM"))
    psU = es.enter_context(tc.tile_pool(name="psumU", bufs=1, space="PSUM"))
    psF = es.enter_context(tc.tile_pool(name="psumF", bufs=1, space="PSUM"))
    psL = es.enter_context(tc.tile_pool(name="psumL", bufs=1, space="PSUM"))
    xtp = es.enter_context(tc.tile_pool(name="xtp", bufs=2))

    # one-time W_u transpose: wT[c, g] chunks (fp32 + bf16 copies)
    for k in range(NKC):
        tr = psF.tile([P, G], F32, tag="fix")
        nc.tensor.transpose(tr[:, :G], w_sb[:, k * P:(k + 1) * P], ident[:G, :G])
        nc.vector.tensor_copy(wT_sb[:, k, :], tr[:, :G])
        nc.scalar.copy(wT_bf[:, k, :], tr[:, :G])

    cache_tiles = {}            # (b, d) -> bf16 slab tile [P, nj, C]
    ystage = {}                 # (b, h) -> f32 tile [P, 2, C]

    def emit_slab(b, d, di, sums_ps):
        slab = stage.tile([P, nj, C], F32, tag="slab")
        nc.sync.dma_start(
            slab[:], x[d, b].rearrange("(p j) c -> p j c", j=nj))
        xbf = cache.tile([P, nj, C], BF16, tag="cslab")
        cache_tiles[(b, d)] = xbf
        # casts all on ACT (two [P, 2C] ops): DVE is reserved for combines
        for j in range(0, nj, 2):
            nc.scalar.copy(xbf[:, j:j + 2, :], slab[:, j:j + 2, :])
        # pair j-reduction on DVE (bf16 2x fast mode) halves the moving
        # columns PE must stream for the indicator column-sums
        jsum = jsump.tile([P, 2, C], BF16, tag="jsum")
        nc.vector.tensor_add(jsum[:, 0, :], xbf[:, 0, :], xbf[:, 1, :])
        nc.vector.tensor_add(jsum[:, 1, :], xbf[:, 2, :], xbf[:, 3, :])
        for h in range(2):
            for jj in range(2):
                nc.tensor.matmul(
                    sums_ps[:, h * 512:(h + 1) * 512],
                    ind_bf[:, d, :],
                    jsum[:, jj, h * 512:(h + 1) * 512],
                    start=(di == 0 and jj == 0),
                    stop=(di == D - 1 and jj == 1),
                )

    def emit_ublock(b, j):
        # uT[g, s-block] = sum_k (wT_k).T @ x7T_k on PE
        x7bf = cache_tiles[(b, D - 1)]
        u_ps = psU.tile([G, P], F32, tag="u")
        for k in range(NKC):
            xt_ps = psT.tile([P, P], BF16, tag="xt")
            nc.tensor.transpose(
                xt_ps[:], x7bf[:, j, k * P:(k + 1) * P], ident_bf[:])
            xt_sb = xtp.tile([P, P], BF16, tag="xt_sb")
            if k % 2 == 0:
                nc.scalar.copy(xt_sb[:], xt_ps[:])
            else:
                nc.vector.tensor_copy(xt_sb[:], xt_ps[:])
            nc.tensor.matmul(
                u_ps[:], wT_bf[:, k, :], xt_sb[:],
                start=(k == 0), stop=(k == NKC - 1))
        nc.scalar.copy(uT_sb[:, b, j, :], u_ps[:])

    def emit_fixup(b, sums_ps):
        # sums -> meanT chunks -> partial keysT; fixup copies on ACT so
        # the AR trigger is never gated by DVE combine backlog
        sums_sb = fxp.tile([D, C], F32, tag="sums_sb")
        nc.scalar.copy(sums_sb[:], sums_ps[:])
        mt_ps = psF.tile([P, NKC * D], F32, tag="fix")
        for k in range(NKC):
            nc.tensor.matmul(
                mt_ps[:, k * D:(k + 1) * D],
                sums_sb[:, k * P:(k + 1) * P], ident[:D, :D],
                is_transpose=True, start=(k == 0), stop=(k == NKC - 1))
        meanT_tmp = xtp.tile([P, NKC * D], F32, tag="mt")
        nc.scalar.copy(meanT_tmp[:], mt_ps[:])
        keys_ps = psF.tile([P, NKC * D], F32, tag="fix")
        for k in range(NKC):
            nc.tensor.matmul(
                keys_ps[:G, :D],
                wT_sb[:, k, :],
                meanT_tmp[:, k * D:(k + 1) * D],
                start=(k == 0), stop=(k == NKC - 1))
        nc.scalar.copy(sumk_sb[:, b, :], keys_ps[:G, :D])
        nc.gpsimd.dma_start(cc_in[b][:], sumk_sb[:, b, :])
        nc.gpsimd.collective_compute(
            "AllReduce", add, replica_groups=[list(range(N_CORES))],
            ins=[cc_in[b].opt()], outs=[cc_out[b].opt()],
        )

    def emit_prelude(b):
        # keysT fetch + logits + softmax for batch b
        nc.gpsimd.dma_start(keysT_sb[:, b, :], cc_out[b][:])
        nc.scalar.copy(keysT_bf[:, b, :], keysT_sb[:, b, :])
        for j in range(nj):
            lg_ps = psL.tile([P, D], F32, tag="lg")
            nc.tensor.matmul(lg_ps[:], uT_sb[:, b, j, :], keysT_bf[:, b, :])
            e_sb = xtp.tile([P, D], F32, tag="e")
            z_sb = xtp.tile([P, 1], F32, tag="z")
            rz_sb = xtp.tile([P, 1], F32, tag="rz")
            nc.scalar.activation(
                e_sb[:], lg_ps[:], mybir.ActivationFunctionType.Exp,
                accum_out=z_sb[:])
            nc.vector.reciprocal(rz_sb[:], z_sb[:])
            nc.scalar.mul(gates_sb[:, b, j, :], e_sb[:], rz_sb[:])

    def emit_chain(b, j):
        # weighted-depth combine for block j, all in bf16 so every op runs
        # in the DVE 2x fast mode (tensor_scalar / tensor_tensor; STT is
        # not fast-mode eligible): 8 products + 7 adds, final add emits
        # f32. bf16 accumulation costs ~0.5% relative — budget is 2e-2.
        # For the tail batch the products run on ACT (idle there), halving
        # the exposed tail combine.
        h, jj = j // 2, j % 2
        n_act = D - 1 if b == B - 1 else 0
        if jj == 0:
            yst_t = ystp.tile([P, 2, C], F32, tag="yst", name=f"yst_{b}_{h}")
            ystage[(b, h)] = yst_t
        yst = ystage[(b, h)]
        acc = apool.tile([P, C], BF16, tag="cacc")
        dorder = [D - 1] + list(range(D - 1))
        for di, d in enumerate(dorder):
            xbf = cache_tiles[(b, d)]
            g = gates_sb[:, b, j, d:d + 1]
            if di == 0:
                nc.vector.tensor_scalar_mul(acc[:], xbf[:, j, :], g)
                continue
            p = ppool.tile([P, C], BF16, tag="prod")
            if di <= n_act:
                nc.scalar.mul(p[:], xbf[:, j, :], g)
            else:
                nc.vector.tensor_scalar_mul(p[:], xbf[:, j, :], g)
            if di < D - 1:
                nc.vector.tensor_add(acc[:], acc[:], p[:])
            else:
                nc.vector.tensor_add(yst[:, jj, :], acc[:], p[:])
        if j == 3:
            for d in range(D):
                del cache_tiles[(b, d)]

    def emit_ywrite(b, h):
        nc.gpsimd.dma_start(
            y[b].rearrange("(p j) c -> p j c", j=nj)[:, 2 * h:2 * h + 2, :],
            ystage.pop((b, h))[:])

    dorder = [D - 1] + list(range(D - 1))
    for b in range(B):
        sums_ps = psS.tile([D, C], F32, tag="sums")
        for si, d in enumerate(dorder):
            emit_slab(b, d, si, sums_ps)
            if 1 <= si <= nj:
                emit_ublock(b, si - 1)
            if b >= 1:
                if si == 5:
                    emit_prelude(b - 1)
                elif si == 6:
                    emit_chain(b - 1, 0)
                    emit_chain(b - 1, 1)
                    emit_ywrite(b - 1, 0)
                elif si == 7:
                    emit_chain(b - 1, 2)
        emit_fixup(b, sums_ps)
        if b >= 1:
            emit_chain(b - 1, 3)
            emit_ywrite(b - 1, 1)
    emit_prelude(B - 1)
    for j in range(nj):
        emit_chain(B - 1, j)
        if j % 2 == 1:
            emit_ywrite(B - 1, j // 2)

    es.close()


def build_nc(s_sh):
    nc = bacc.Bacc("TRN2", target_bir_lowering=False, debug=False,
                   num_devices=N_CORES)
    x_ap = nc.dram_tensor("x", [D, B, s_sh, C], F32, kind="ExternalInput").ap()
    w_ap = nc.dram_tensor("w", [G, C], F32, kind="ExternalInput").ap()
    y_ap = nc.dram_tensor("y", [B, s_sh, C], F32, kind="ExternalOutput").ap()
    with tile.TileContext(nc) as tc:
        build_body(tc, x_ap, w_ap, y_ap, s_sh)
    nc.compile()
    return nc


_NC_CACHE = {}


def _get_nc(s_sh):
    if s_sh not in _NC_CACHE:
        _NC_CACHE[s_sh] = build_nc(s_sh)
    return _NC_CACHE[s_sh]


def run(cached_states, W_u, trace=False, trace_cores=None):
    s_sh = S // N_CORES
    nc = _get_nc(s_sh)
    xs = np.asarray(cached_states, dtype=np.float32)
    ws = np.ascontiguousarray(np.asarray(W_u, dtype=np.float32))
    in_maps = []
    for i in range(N_CORES):
        sh = np.ascontiguousarray(xs[:, :, i * s_sh:(i + 1) * s_sh, :])
        in_maps.append({"x": sh, "w": ws})
    res = bass_utils.run_bass_kernel_spmd(
        nc, in_maps, core_ids=list(range(N_CORES)), trace=trace,
        trace_cores=trace_cores)
    out = np.empty((B, S, C), np.float32)
    for i in range(N_CORES):
        out[:, i * s_sh:(i + 1) * s_sh, :] = res.results[i]["y"]
    return out, res


def kernel(cached_states, W_u):
    out, _ = run(cached_states, W_u)
    return out
